# revision 1
# baseline (speedup 1.0000x reference)
"""Trainium2 Bass kernel for BayesianChangePointDetector (segment_reduce).

Contract: kernel(**inputs) takes FULL inputs (x:[128,8192,32] f32, plus 3
scalar prior params) and returns the FULL [128] f32 output. Internally the
batch dim is sharded across 8 NeuronCores (16 rows each, pure data parallel,
no collectives), each core runs the same Bass/Tile program, and the host
concatenates the 8 per-core [16] outputs.

Per-core layout: partition p in [0,128) owns t in [64p, 64p+64); the free dim
is (b, u) with b in [0,16) batch rows and u in [0,64). The heavy pass is a
single DVE reduce over N=32; prefix sums use the native tensor_tensor_scan
plus a cross-partition carry fixed up with a triangular-ones matmul on PE.
"""

import sys

if "/opt/trn_rl_repo" not in sys.path:
    sys.path.insert(0, "/opt/trn_rl_repo")

import math
from contextlib import ExitStack

import numpy as np

import concourse.bass as bass
import concourse.tile as tile
from concourse import mybir

F32 = mybir.dt.float32
AF = mybir.ActivationFunctionType
ALU = mybir.AluOpType
AX = mybir.AxisListType

B, T, N = 128, 8192, 32
NCORES = 8
BL = B // NCORES  # 16 batch rows per core
P = 128           # partitions = t-blocks
U = T // P        # 64 t's per partition
BC = 4            # batch rows per processing chunk
NCHUNK = BL // BC
UF = 32           # u-range whose n-fold (32->16) runs on gpsimd
UF_SCHED = [16, 32, 32, 32]  # per-chunk fold; small for chunk0 (early DVE start)
CHUNK_SIZES = [4, 4, 4, 4]   # batch rows per chunk (uniform won all model sweeps)
XP_BUFS = 2       # x staging double-buffer depth
WK_BUFS = 3       # per-chunk work tile depth
NS = 32           # scalar-slot count
NEG = -1.0e30

# near-end threshold: P_split > 6553  <=>  g >= 6553 (g = P_split-1 = 64p+u)
NE_P0 = 6553 // U          # 102
NE_U0 = 6553 - NE_P0 * U   # 25
# valid candidates: P_split in [16, 8176) <=> g in [15, 8175)
LO_INV_U = 15              # g<15 -> p==0, u<15 invalid
HI_INV_U = 8174 - 127 * U + 1  # g>8174 -> p==127, u>=47 invalid


def build_body(ctx, tc, x, params, utc, idc, out, pm_zero=False):
    nc = tc.nc
    pers = ctx.enter_context(tc.tile_pool(name="pers", bufs=1))
    xp = ctx.enter_context(tc.tile_pool(name="xp", bufs=XP_BUFS))
    wk = ctx.enter_context(tc.tile_pool(name="wk", bufs=WK_BUFS))
    psp = ctx.enter_context(tc.tile_pool(name="psp", bufs=2, space="PSUM"))
    ps1 = ctx.enter_context(tc.tile_pool(name="ps1", bufs=1, space="PSUM"))

    # ---------- small const DMAs first (same SP ring, FIFO ahead of x),
    # then x DMAs split in u-halves for earlier reduce start ----------
    ut_t = pers.tile([P, P], F32)     # strictly-upper triangular ones (q<m)
    ones_t = pers.tile([P, P], F32)   # all-ones
    id_t = pers.tile([P, P], F32)     # identity (PE transpose)
    gt = pers.tile([P, U], F32)       # g = 64p+u
    ptile = pers.tile([P, 3], F32)
    nc.sync.dma_start(ptile[:], params[:])
    nc.gpsimd.memset(ones_t[:], 1.0)
    gti = pers.tile([P, U], mybir.dt.int32)
    nc.gpsimd.iota(gti[:], [[1, U]], base=0, channel_multiplier=U)
    nc.vector.tensor_copy(gt[:], gti[:])

    chunks = []
    o = 0
    for c in CHUNK_SIZES:
        chunks.append((o, c))
        o += c
    assert o == BL
    xts = []
    for ci, (bs, bc) in enumerate(chunks):
        xt = xp.tile([P, bc, U, N], F32, tag="xt")
        src = x[bs : bs + bc].rearrange("b (p u) n -> p b u n", p=P)
        if ci == 0:
            QU = U // 4
            for q in range(4):
                nc.sync.dma_start(
                    xt[:, :, q * QU : (q + 1) * QU, :],
                    src[:, :, q * QU : (q + 1) * QU, :],
                )
        else:
            HU = U // 2
            nc.sync.dma_start(xt[:, :, 0:HU, :], src[:, :, 0:HU, :])
            nc.sync.dma_start(xt[:, :, HU:U, :], src[:, :, HU:U, :])
        if ci == 0:
            # needed from the first carry matmul / finale only; land behind chunk0
            nc.sync.dma_start(ut_t[:], utc[:])
            nc.sync.dma_start(id_t[:], idc[:])
        xts.append(xt)

    # scalar slots, computed redundantly on all 128 partitions
    sv = pers.tile([P, NS], F32)
    tmp = pers.tile([P, 8], F32)

    def s(i):
        return sv[:, i : i + 1]

    def tm(i):
        return tmp[:, i : i + 1]

    # ---------- scalar prep on partition 0 ----------
    # slots: 0 pm, 1 inv_nv, 2 inv_pv, 3 neg_inv_nv, 4 zRb, 5 k, 6 c,
    # 7 -kq/2, 8 k^2/2, 9 kq/2, 10 c*k, 11 c^2/2, 12 sc, 13 pvW,
    # 14 L2pinv, 15 Lpv, 16 LpvW, 17 8192*inv_nv, 18 inv_nv/8192,
    # 19 bfWc, 20 pv, 21 nv, 22 pm^2*inv_pv, 23 -4096*L2pinv
    # softplus(x) = ln(1 + exp(x)); Exp+Ln share one ACT table set
    nc.scalar.activation(tm(0), ptile[:, 1:2], AF.Exp)
    nc.vector.tensor_scalar_add(tm(0), tm(0), 1.0)
    nc.scalar.activation(s(20), tm(0), AF.Ln)
    nc.scalar.activation(tm(1), ptile[:, 2:3], AF.Exp)
    nc.vector.tensor_scalar_add(tm(1), tm(1), 1.0)
    nc.scalar.activation(s(21), tm(1), AF.Ln)
    nc.vector.tensor_copy(s(0), ptile[:, 0:1])
    nc.vector.reciprocal(s(1), s(21))
    nc.vector.reciprocal(s(2), s(20))
    nc.vector.tensor_scalar_mul(s(3), s(1), -1.0)
    nc.vector.tensor_scalar(s(4), s(1), 8191.0, s(2), ALU.mult, ALU.add)
    nc.vector.tensor_scalar_mul(s(5), s(1), 1.0 / 32.0)
    nc.vector.tensor_mul(s(6), s(0), s(2))
    nc.vector.tensor_scalar_mul(s(7), s(1), -0.5 / 1024.0)
    nc.vector.tensor_scalar_mul(s(9), s(1), 0.5 / 1024.0)
    nc.vector.tensor_mul(tm(0), s(5), s(5))
    nc.vector.tensor_scalar_mul(s(8), tm(0), 0.5)
    nc.vector.tensor_mul(s(10), s(6), s(5))
    nc.vector.tensor_mul(tm(1), s(6), s(6))
    nc.vector.tensor_scalar_mul(s(11), tm(1), 0.5)
    nc.scalar.activation(s(14), s(21), AF.Ln, scale=2.0 * math.pi)
    nc.scalar.activation(s(15), s(20), AF.Ln)
    nc.vector.tensor_scalar_mul(s(17), s(1), 8192.0)
    nc.vector.tensor_scalar(tm(2), s(1), 8192.0, s(2), ALU.mult, ALU.add)
    nc.vector.reciprocal(s(13), tm(2))
    nc.scalar.activation(s(16), s(13), AF.Ln)
    nc.vector.tensor_scalar_mul(s(18), s(1), 1.0 / 8192.0)
    nc.vector.tensor_mul(tm(3), s(0), s(0))
    nc.vector.tensor_mul(s(22), tm(3), s(2))
    nc.vector.tensor_scalar_mul(s(23), s(14), -4096.0)
    nc.vector.tensor_sub(tm(4), s(23), s(15))
    nc.vector.tensor_sub(s(12), tm(4), s(22))
    nc.vector.tensor_sub(tm(5), s(16), s(15))
    nc.vector.tensor_scalar_mul(tm(5), tm(5), 0.5)
    nc.vector.tensor_add(tm(6), s(23), tm(5))
    nc.vector.tensor_scalar_mul(tm(7), s(22), -0.5)
    nc.vector.tensor_add(s(19), tm(6), tm(7))

    def sb(i, np_=P, p0=0):
        return sv[p0 : p0 + np_, i : i + 1]

    # ---------- per-candidate coefficient vectors [P, U] ----------
    nf = pers.tile([P, U], F32)
    nc.vector.tensor_scalar_add(nf[:], gt[:], 1.0)
    zL = pers.tile([P, U], F32)
    nc.vector.tensor_scalar(zL[:], nf[:], sb(1), sb(2), ALU.mult, ALU.add)
    pvnL = pers.tile([P, U], F32)
    nc.vector.reciprocal(pvnL[:], zL[:])
    zR = pers.tile([P, U], F32)
    nc.vector.tensor_scalar(zR[:], gt[:], sb(3), sb(4), ALU.mult, ALU.add)
    pvnR = pers.tile([P, U], F32)
    nc.vector.reciprocal(pvnR[:], zR[:])
    lpvnL = pers.tile([P, U], F32)
    nc.scalar.activation(lpvnL[:], pvnL[:], AF.Ln)
    lpvnR = pers.tile([P, U], F32)
    nc.scalar.activation(lpvnR[:], pvnR[:], AF.Ln)
    kc2 = pers.tile([P, U], F32)
    nc.vector.tensor_add(kc2[:], lpvnL[:], lpvnR[:])

    nRf = pers.tile([P, U], F32)
    nc.vector.tensor_scalar(nRf[:], gt[:], -1.0, 8191.0, ALU.mult, ALU.add)
    gc = pers.tile([P, U], F32)
    nc.vector.tensor_scalar_max(gc[:], gt[:], 1.0)
    inv_n1 = pers.tile([P, U], F32)
    nc.vector.reciprocal(inv_n1[:], gc[:])
    nR1c = pers.tile([P, U], F32)
    nc.vector.tensor_scalar(nR1c[:], gt[:], -1.0, 8190.0, ALU.mult, ALU.add)
    nc.vector.tensor_scalar_max(nR1c[:], nR1c[:], 1.0)
    inv_nR1 = pers.tile([P, U], F32)
    nc.vector.reciprocal(inv_nR1[:], nR1c[:])
    inv_n = pers.tile([P, U], F32)
    nc.vector.reciprocal(inv_n[:], nf[:])
    inv_nR = pers.tile([P, U], F32)
    nRc = pers.tile([P, U], F32)
    nc.vector.tensor_scalar_max(nRc[:], nRf[:], 1.0)
    nc.vector.reciprocal(inv_nR[:], nRc[:])

    n_n1 = pers.tile([P, U], F32)
    nc.vector.tensor_mul(n_n1[:], nf[:], inv_n1[:])
    nR_nR1 = pers.tile([P, U], F32)
    nc.vector.tensor_mul(nR_nR1[:], nRf[:], inv_nR1[:])
    i_nn1 = pers.tile([P, U], F32)
    nc.vector.tensor_mul(i_nn1[:], inv_n[:], inv_n1[:])
    i_nRnR1 = pers.tile([P, U], F32)
    nc.vector.tensor_mul(i_nRnR1[:], inv_nR[:], inv_nR1[:])

    CBL = pers.tile([P, U], F32)
    nc.scalar.activation(CBL[:], n_n1[:], AF.Copy, scale=sb(7))
    CBR = pers.tile([P, U], F32)
    nc.scalar.activation(CBR[:], nR_nR1[:], AF.Copy, scale=sb(7))
    # CA2L = 0.5*kq*i_nn1 + 0.5*k^2*pvnL
    CA2L = pers.tile([P, U], F32)
    q1 = pers.tile([P, U], F32)
    nc.scalar.activation(q1[:], pvnL[:], AF.Copy, scale=sb(8))
    q2 = pers.tile([P, U], F32)
    nc.scalar.activation(q2[:], i_nn1[:], AF.Copy, scale=sb(9))
    nc.vector.tensor_add(CA2L[:], q1[:], q2[:])
    CA2R = pers.tile([P, U], F32)
    q1b = pers.tile([P, U], F32)
    nc.scalar.activation(q1b[:], pvnR[:], AF.Copy, scale=sb(8))
    q2b = pers.tile([P, U], F32)
    nc.scalar.activation(q2b[:], i_nRnR1[:], AF.Copy, scale=sb(9))
    nc.vector.tensor_add(CA2R[:], q1b[:], q2b[:])
    CAL = pers.tile([P, U], F32)
    nc.scalar.activation(CAL[:], pvnL[:], AF.Copy, scale=sb(10))
    CAR = pers.tile([P, U], F32)
    nc.scalar.activation(CAR[:], pvnR[:], AF.Copy, scale=sb(10))
    Cc = pers.tile([P, U], F32)
    p12 = pers.tile([P, U], F32)
    nc.vector.tensor_add(p12[:], pvnL[:], pvnR[:])
    cc1 = pers.tile([P, U], F32)
    nc.scalar.activation(cc1[:], p12[:], AF.Copy, scale=sb(11))
    cct = pers.tile([P, U], F32)
    nc.vector.tensor_scalar(cct[:], kc2[:], 0.5, sb(12), ALU.mult, ALU.add)
    nc.vector.tensor_add(Cc[:], cc1[:], cct[:])
    # bake the invalid-candidate mask into Cc: bf = ... + Cc ~ -1e30 there.
    # valid g in [15, 8175); compute via two is_ge comparisons on gt.
    mlo = pers.tile([P, U], F32)
    nc.vector.tensor_scalar(mlo[:], gt[:], 14.5, NEG, ALU.is_lt, ALU.mult)
    mhi = pers.tile([P, U], F32)
    nc.vector.tensor_scalar(mhi[:], gt[:], 8174.5, NEG, ALU.is_ge, ALU.mult)
    nc.vector.tensor_add(Cc[:], Cc[:], mlo[:])
    nc.vector.tensor_add(Cc[:], Cc[:], mhi[:])
    # near-end 0/1 mask (g >= 6553)
    nemask = pers.tile([P, U], F32)
    nc.vector.tensor_scalar(nemask[:], gt[:], 6552.5, None, ALU.is_ge)

    # ---------- persistent accumulators ----------
    bund = pers.tile([P, 80], F32)  # [0:16) rmax | [16:32) Zp | [32:48) En | [48:64) At | [64:80) Bt
    zeros = pers.tile([P, max(CHUNK_SIZES) * U], F32)
    nc.gpsimd.memset(zeros[:], 0.0)

    # ---------- finale tiles (filled incrementally from the last chunk) ----------
    Tall = ps1.tile([BL, 5, P], F32)
    Tm = Tall[:, 0, :]
    Tz = Tall[:, 1, :]
    Te = Tall[:, 2, :]
    Ta = Tall[:, 3, :]
    Tb = Tall[:, 4, :]
    M16 = pers.tile([BL, 1], F32)
    d = pers.tile([BL, P], F32)
    w = pers.tile([BL, P], F32)
    bfW = pers.tile([BL, 1], F32)
    sig = pers.tile([BL, 1], F32)

    def trace_bfw():
        # whole-window log marginal: only needs At/Bt (bund cols 48:80)
        nc.tensor.transpose(Ta, bund[:, 48 : 48 + BL], id_t[:])
        nc.tensor.transpose(Tb, bund[:, 64 : 64 + BL], id_t[:])
        At16 = Ta[:, 0:1]
        Bt16 = Tb[:, 0:1]
        t1 = pers.tile([BL, 1], F32)
        nc.scalar.activation(t1[:], At16, AF.Square, scale=1.0 / 32.0)
        v2 = pers.tile([BL, 1], F32)
        nc.vector.tensor_scalar_mul(v2[:], t1[:], 1.0 / 8192.0)
        vW = pers.tile([BL, 1], F32)
        nc.vector.scalar_tensor_tensor(
            vW[:], Bt16, 1.0 / 1024.0, v2[:], ALU.mult, ALU.subtract
        )
        nc.vector.tensor_scalar(vW[:], vW[:], 1.0 / 8191.0, 1.0e-8, ALU.mult, ALU.max)
        term1 = pers.tile([BL, 1], F32)
        nc.vector.tensor_scalar_mul(term1[:], vW[:], sb(17, BL))
        term2 = pers.tile([BL, 1], F32)
        nc.vector.tensor_scalar_mul(term2[:], t1[:], sb(18, BL))
        uu = pers.tile([BL, 1], F32)
        nc.scalar.activation(uu[:], At16, AF.Identity, bias=sb(6, BL), scale=sb(5, BL))
        u2 = pers.tile([BL, 1], F32)
        nc.scalar.activation(u2[:], uu[:], AF.Square)
        term3 = pers.tile([BL, 1], F32)
        nc.vector.tensor_scalar_mul(term3[:], u2[:], sb(13, BL))
        tsum = pers.tile([BL, 1], F32)
        nc.vector.tensor_add(tsum[:], term1[:], term2[:])
        nc.vector.tensor_sub(tsum[:], tsum[:], term3[:])
        nc.vector.tensor_scalar(bfW[:], tsum[:], -0.5, sb(19, BL), ALU.mult, ALU.add)

    def trace_maxw():
        # bund[:,0:16] holds -rmax; M16 = max(rmax) = -min(-rmax) via negate
        nc.tensor.transpose(Tm, bund[:, 0:BL], id_t[:])
        nc.vector.tensor_reduce(M16[:], Tm, AX.X, ALU.min, negate=True)
        # d = rmax_p - M16 = (-Tm) - M16
        nc.vector.tensor_scalar(d[:], Tm, -1.0, M16[:], ALU.mult, ALU.subtract)
        nc.scalar.activation(w[:], d[:], AF.Exp)
        sigin = pers.tile([BL, 1], F32)
        nc.vector.tensor_sub(sigin[:], M16[:], bfW[:])
        nc.scalar.activation(sig[:], sigin[:], AF.Sigmoid)

    # ---------- per-chunk pipeline ----------
    # trace each chunk's gpsimd fold ahead of the previous chunk's Pool work
    # (in-order engine streams: otherwise the fold queues behind the adds)
    xhs = [None] * len(chunks)

    def ufof(ci):
        return UF_SCHED[ci] if UF_SCHED is not None else UF

    def trace_fold(ci):
        uf = ufof(ci)
        bc = chunks[ci][1]
        if uf > 0:
            xh = wk.tile([P, bc, uf, 16], F32, tag="xh")
            nc.gpsimd.tensor_add(
                xh[:], xts[ci][:, :, 0:uf, 0:16], xts[ci][:, :, 0:uf, 16:32]
            )
            xhs[ci] = xh

    trace_fold(0)
    for ci, (bs, bc) in enumerate(chunks):
        xt = xts[ci]
        last = ci == len(chunks) - 1
        if not last:
            trace_fold(ci + 1)

        uf = ufof(ci)
        sr = wk.tile([P, bc, U], F32)
        if uf > 0:
            if ci == 0 and uf == 16:
                # quartered first chunk: reduce each raw quarter as it lands
                for q in range(1, 4):
                    nc.vector.tensor_reduce(
                        sr[:, :, q * 16 : (q + 1) * 16],
                        xt[:, :, q * 16 : (q + 1) * 16, :],
                        AX.X,
                        ALU.add,
                    )
            elif uf < U:
                nc.vector.tensor_reduce(
                    sr[:, :, uf:U], xt[:, :, uf:U, :], AX.X, ALU.add
                )
            nc.vector.tensor_reduce(sr[:, :, 0:uf], xhs[ci][:], AX.X, ALU.add)
        else:
            HU = U // 2
            nc.vector.tensor_reduce(
                sr[:, :, 0:HU], xt[:, :, 0:HU, :], AX.X, ALU.add
            )
            nc.vector.tensor_reduce(
                sr[:, :, HU:U], xt[:, :, HU:U, :], AX.X, ALU.add
            )
        sq = wk.tile([P, bc, U], F32)
        nc.scalar.activation(sq[:], sr[:], AF.Square)

        A = wk.tile([P, bc, U], F32)
        nc.vector.tensor_tensor_scan(
            A[:].rearrange("p b u -> p (b u)"),
            sr[:].rearrange("p b u -> p (b u)"),
            zeros[:, 0 : bc * U],
            0.0,
            ALU.add,
            ALU.add,
        )
        Bt_ = wk.tile([P, bc, U], F32)
        nc.vector.tensor_tensor_scan(
            Bt_[:].rearrange("p b u -> p (b u)"),
            sq[:].rearrange("p b u -> p (b u)"),
            zeros[:, 0 : bc * U],
            0.0,
            ALU.add,
            ALU.add,
        )

        # carry fix: rowprev, chunk totals, triangular matmul
        rv = wk.tile([P, 2 * bc], F32)  # [0:bc) rvA | [bc:2BC) rvB
        nc.gpsimd.memset(rv[:, 0:1], 0.0)
        nc.gpsimd.memset(rv[:, bc : bc + 1], 0.0)
        nc.vector.tensor_copy(rv[:, 1:bc], A[:, 0 : bc - 1, U - 1])
        nc.vector.tensor_copy(rv[:, bc + 1 : 2 * bc], Bt_[:, 0 : bc - 1, U - 1])
        ct = wk.tile([P, 2 * bc], F32)
        nc.vector.tensor_sub(ct[:, 0:bc], A[:, :, U - 1], rv[:, 0:bc])
        nc.vector.tensor_sub(ct[:, bc : 2 * bc], Bt_[:, :, U - 1], rv[:, bc : 2 * bc])
        g_ps = psp.tile([P, 2 * bc], F32)
        nc.tensor.matmul(g_ps[:], ut_t[:], ct[:])
        tot_ps = psp.tile([P, 2 * bc], F32)
        nc.tensor.matmul(tot_ps[:], ones_t[:], ct[:])
        off = wk.tile([P, 2 * bc], F32)
        nc.vector.tensor_sub(off[:], g_ps[:], rv[:])

        offA_b = off[:, 0:bc].unsqueeze(2).broadcast_to([P, bc, U])
        offB_b = off[:, bc : 2 * bc].unsqueeze(2).broadcast_to([P, bc, U])
        nc.vector.tensor_add(A[:], A[:], offA_b)
        nc.vector.tensor_add(Bt_[:], Bt_[:], offB_b)

        # At/Bt to SBUF (bund doubles as the staging buffer; gpsimd can't read PSUM)
        nc.scalar.copy(bund[:, 48 + bs : 48 + bs + bc], tot_ps[:, 0:bc])
        nc.scalar.copy(bund[:, 64 + bs : 64 + bs + bc], tot_ps[:, bc : 2 * bc])
        if last:
            trace_bfw()
        At_b = (
            bund[:, 48 + bs : 48 + bs + bc].unsqueeze(2).broadcast_to([P, bc, U])
        )
        Btot_b = (
            bund[:, 64 + bs : 64 + bs + bc].unsqueeze(2).broadcast_to([P, bc, U])
        )
        AR = wk.tile([P, bc, U], F32)
        nc.gpsimd.tensor_sub(AR[:], At_b, A[:])
        BR = wk.tile([P, bc, U], F32)
        nc.gpsimd.tensor_sub(BR[:], Btot_b, Bt_[:])

        A2 = wk.tile([P, bc, U], F32)
        nc.scalar.activation(A2[:], A[:], AF.Square)
        AR2 = wk.tile([P, bc, U], F32)
        nc.scalar.activation(AR2[:], AR[:], AF.Square)

        def cb(t):
            return t[:].unsqueeze(1).broadcast_to([P, bc, U])

        bf = wk.tile([P, bc, U], F32)
        p2 = wk.tile([P, bc, U], F32)
        p3 = wk.tile([P, bc, U], F32)
        p5 = wk.tile([P, bc, U], F32)
        p6 = wk.tile([P, bc, U], F32)
        nc.vector.tensor_mul(p2[:], A2[:], cb(CA2L))
        nc.vector.tensor_mul(p3[:], Bt_[:], cb(CBL))
        nc.vector.tensor_mul(p5[:], AR2[:], cb(CA2R))
        (nc.vector if last else nc.gpsimd).tensor_mul(p6[:], BR[:], cb(CBR))
        if pm_zero:
            # c = pm/pv = 0 -> the A and AR linear terms vanish
            eng1 = nc.vector if last else nc.gpsimd
            eng1.tensor_add(p2[:], p2[:], p3[:])
            nc.vector.tensor_add(p5[:], p5[:], p6[:])
            eng1.tensor_add(p2[:], p2[:], cb(Cc))
            nc.vector.tensor_add(bf[:], p2[:], p5[:])
        else:
            p1 = wk.tile([P, bc, U], F32)
            p4 = wk.tile([P, bc, U], F32)
            nc.vector.tensor_mul(p1[:], A[:], cb(CAL))
            nc.vector.tensor_mul(p4[:], AR[:], cb(CAR))
            nc.gpsimd.tensor_add(p1[:], p1[:], p2[:])
            nc.vector.tensor_add(p3[:], p3[:], p4[:])
            nc.gpsimd.tensor_add(p5[:], p5[:], p6[:])
            nc.gpsimd.tensor_add(p1[:], p1[:], cb(Cc))
            nc.vector.tensor_add(p3[:], p3[:], p5[:])
            nc.vector.tensor_add(bf[:], p1[:], p3[:])

        # per-(p,b) NEGATED max (negate=True -> -max), exp with shift, partial sums
        nc.vector.tensor_reduce(
            bund[:, bs : bs + bc], bf[:], AX.X, ALU.max, negate=True
        )
        if last:
            trace_maxw()
        e = wk.tile([P, bc, U], F32)
        for b in range(bc):
            nc.scalar.activation(
                e[:, b, :],
                bf[:, b, :],
                AF.Exp,
                bias=bund[:, bs + b : bs + b + 1],
                accum_out=bund[:, 16 + bs + b : 17 + bs + b],
            )
        # near-end partial sums: sum_u e * nemask
        en = wk.tile([P, bc, U], F32)
        (nc.vector if last else nc.gpsimd).tensor_mul(en[:], e[:], cb(nemask))
        nc.vector.tensor_reduce(
            bund[:, 32 + bs : 32 + bs + bc], en[:], AX.X, ALU.add
        )

    # ---------- finale: Zb/Nb combine (Tm/Ta/Tb + bfW already traced) ----------
    nc.tensor.transpose(Tz, bund[:, 16 : 16 + BL], id_t[:])
    nc.tensor.transpose(Te, bund[:, 32 : 32 + BL], id_t[:])
    wz = pers.tile([BL, P], F32)
    Zb = pers.tile([BL, 1], F32)
    nc.vector.scalar_tensor_tensor(
        wz[:], w[:], 1.0, Tz, ALU.mult, ALU.mult, accum_out=Zb[:]
    )
    wn = pers.tile([BL, P], F32)
    Nb = pers.tile([BL, 1], F32)
    nc.vector.scalar_tensor_tensor(
        wn[:], w[:], 1.0, Te, ALU.mult, ALU.mult, accum_out=Nb[:]
    )
    invZ = pers.tile([BL, 1], F32)
    nc.vector.reciprocal(invZ[:], Zb[:])
    ratio = pers.tile([BL, 1], F32)
    nc.vector.tensor_mul(ratio[:], Nb[:], invZ[:])
    outv = pers.tile([BL, 1], F32)
    nc.vector.tensor_mul(outv[:], sig[:], ratio[:])
    nc.sync.dma_start(out[:], outv[:])


def host_consts():
    ut = np.triu(np.ones((P, P), np.float32), 1)
    ident = np.eye(P, dtype=np.float32)
    return ut, ident


def split_multi_waits(nc):
    """Walrus in this container allows one sync wait per instruction; move
    extra waits onto same-engine NOPs inserted immediately before."""
    import bass_rust

    nid = [0]
    for f in nc.m.functions:
        for b in f.blocks:
            insts = b.instructions
            i = 0
            while i < len(insts):
                ins = insts[i]
                si = ins.sync_info
                if si is not None and si.on_wait is not None and len(si.on_wait) > 1:
                    waits = list(si.on_wait)
                    for w in waits[:-1]:
                        nop = mybir.InstNoOp(
                            name=f"I-waitsplit-{nid[0]}", ins=[], outs=[]
                        )
                        nid[0] += 1
                        nop.engine = ins.engine
                        nop.sync_info = bass_rust.SyncInfo(
                            on_wait=[w], on_update=[]
                        )
                        insts.insert(i, nop)
                        i += 1
                    si.on_wait = waits[-1:]
                i += 1


_NC_CACHE = {}


def build_nc(split=True, reps=1, pm_zero=False):
    global _NC_CACHE
    key = (split, reps, pm_zero)
    if key in _NC_CACHE:
        return _NC_CACHE[key]
    nc = bass.Bass()
    x = nc.declare_dram_parameter("x", [BL, T, N], F32, isOutput=False)
    params = nc.declare_dram_parameter("params", [P, 3], F32, isOutput=False)
    utc = nc.declare_dram_parameter("utc", [P, P], F32, isOutput=False)
    idc = nc.declare_dram_parameter("idc", [P, P], F32, isOutput=False)
    out = nc.declare_dram_parameter("out", [BL, 1], F32, isOutput=True)
    with tile.TileContext(nc) as tc:
        for _ in range(reps):
            with ExitStack() as ctx:
                build_body(
                    ctx, tc, x[:], params[:], utc[:], idc[:],
                    out[:], pm_zero=pm_zero,
                )
    if split:
        split_multi_waits(nc)
    _NC_CACHE[key] = nc
    return nc


def make_in_maps(x, prior_mean, prior_var, noise_var):
    x = np.ascontiguousarray(np.asarray(x, dtype=np.float32))
    params = np.tile(
        np.array(
            [[float(prior_mean[0]), float(prior_var[0]), float(noise_var[0])]],
            dtype=np.float32,
        ),
        (P, 1),
    )
    ut, ident = host_consts()
    in_maps = []
    for c in range(NCORES):
        in_maps.append(
            {
                "x": x[c * BL : (c + 1) * BL],
                "params": params,
                "utc": ut,
                "idc": ident,
            }
        )
    return in_maps


def kernel(x, prior_mean, prior_var, noise_var):
    from concourse.bass_utils import run_bass_kernel_spmd

    in_maps = make_in_maps(x, prior_mean, prior_var, noise_var)
    nc = build_nc(pm_zero=(float(np.asarray(prior_mean).reshape(-1)[0]) == 0.0))
    res = run_bass_kernel_spmd(nc, in_maps, list(range(NCORES)))
    outs = [np.asarray(res.results[c]["out"]).reshape(BL) for c in range(NCORES)]
    return np.concatenate(outs).astype(np.float32)



# revision 5
# speedup vs baseline: 1.2561x; 1.2561x over previous
"""Trainium2 Bass kernel for BayesianChangePointDetector (segment_reduce).

Contract: kernel(**inputs) takes FULL inputs (x:[128,8192,32] f32, plus 3
scalar prior params) and returns the FULL [128] f32 output. The batch dim is
sharded across 8 NeuronCores (16 rows each, pure data parallel, no
collectives); the host concatenates the 8 per-core [16] outputs.

Fast path (pm == 0, the shipped configuration): x is staged to the device in
fp16 (host-side cast; quantization error ~7e-4 against the 2e-2 gate), which
halves the HBM stream to ~23.3us/core. The N=32 feature reduce runs as an
fp16 pairwise fold tree on DVE (2x packed-mode), prefix sums use
tensor_tensor_scan (fp32 state) with the cross-partition/row carry fixed via
a triangular-ones matmul on PE, and the Bayes-factor assembly is fused into
per-row scalar_tensor_tensor/tensor_scalar ops spread across DVE/Pool/Act so
every engine stays under the per-chunk DMA time. bf is shifted by the
expected whole-window log-marginal (slot algebra) so exp needs no per-row max
bias; the data-dependent residual enters only the final sigmoid.

A general-pm fallback keeps the original f32 kernel.
"""

import sys

if "/opt/trn_rl_repo" not in sys.path:
    sys.path.insert(0, "/opt/trn_rl_repo")

import math
from contextlib import ExitStack

import numpy as np

import concourse.bass as bass
import concourse.tile as tile
from concourse import mybir

F32 = mybir.dt.float32
F16 = mybir.dt.float16
AF = mybir.ActivationFunctionType
ALU = mybir.AluOpType
AX = mybir.AxisListType

B, T, N = 128, 8192, 32
NCORES = 8
BL = B // NCORES  # 16 batch rows per core
P = 128           # partitions = t-blocks
U = T // P        # 64 t's per partition
NS = 32
NEG = -1.0e30

# fast-path batch chunking (rows per chunk); last chunk small for short tail
CHUNKS_FAST = [5, 5, 4, 2]

# near-end threshold: mask P_split > 6553  <=>  g >= 6553 (g = 64p+u)
NE_P0 = 6553 // U          # 102
NE_U0 = 6553 - NE_P0 * U   # 25


def build_body_fast(ctx, tc, x, params, utc, idc, out):
    """pm == 0 path. x is fp16 [BL, T, N]."""
    nc = tc.nc
    pers = ctx.enter_context(tc.tile_pool(name="pers", bufs=1))
    xp = ctx.enter_context(tc.tile_pool(name="xp", bufs=len(CHUNKS_FAST)))
    wk = ctx.enter_context(tc.tile_pool(name="wk", bufs=3))
    psp = ctx.enter_context(tc.tile_pool(name="psp", bufs=2, space="PSUM"))
    ps1 = ctx.enter_context(tc.tile_pool(name="ps1", bufs=1, space="PSUM"))

    # ---------- DMAs: params first, then x chunks (chunk0 split in u-quarters
    # for an early fold start), ut/id behind chunk0 ----------
    ptile = pers.tile([P, 3], F32)
    nc.sync.dma_start(ptile[:], params[:])

    chunks = []
    o = 0
    for c in CHUNKS_FAST:
        chunks.append((o, c))
        o += c
    assert o == BL

    ut_t = pers.tile([P, P], F32)
    id_t = pers.tile([P, P], F32)
    ones_t = pers.tile([P, P], F32)
    nc.gpsimd.memset(ones_t[:], 1.0)

    xts = []
    for ci, (bs, bc) in enumerate(chunks):
        xt = xp.tile([P, bc, U, N], F16, tag="xt")
        src = x[bs : bs + bc].rearrange("b (p u) n -> p b u n", p=P)
        if ci == 0:
            QU = U // 4
            for q in range(4):
                nc.sync.dma_start(
                    xt[:, :, q * QU : (q + 1) * QU, :],
                    src[:, :, q * QU : (q + 1) * QU, :],
                )
            nc.sync.dma_start(ut_t[:], utc[:])
            nc.sync.dma_start(id_t[:], idc[:])
        else:
            HU = U // 2
            nc.sync.dma_start(xt[:, :, 0:HU, :], src[:, :, 0:HU, :])
            nc.sync.dma_start(xt[:, :, HU:U, :], src[:, :, HU:U, :])
        xts.append(xt)

    # ---------- scalar slots (computed on all partitions) ----------
    sv = pers.tile([P, NS], F32)
    tmp = pers.tile([P, 8], F32)

    def s(i):
        return sv[:, i : i + 1]

    def tm(i):
        return tmp[:, i : i + 1]

    def sb(i, np_=P, p0=0):
        return sv[p0 : p0 + np_, i : i + 1]

    # slots: 1 inv_nv, 2 inv_pv, 3 -inv_nv, 4 zRb, 5 k, 7 -inv_nv/2048,
    # 8 k^2/2, 9 inv_nv/2048, 13 pvW, 14 L2pinv, 15 Lpv, 16 LpvW,
    # 17 8192*inv_nv, 18 inv_nv/8192, 19 bfWc, 20 pv, 21 nv, 24 s19-LWC=128*inv_nv
    nc.scalar.activation(tm(0), ptile[:, 1:2], AF.Exp)
    nc.vector.tensor_scalar_add(tm(0), tm(0), 1.0)
    nc.scalar.activation(s(20), tm(0), AF.Ln)
    nc.scalar.activation(tm(1), ptile[:, 2:3], AF.Exp)
    nc.vector.tensor_scalar_add(tm(1), tm(1), 1.0)
    nc.scalar.activation(s(21), tm(1), AF.Ln)
    nc.vector.reciprocal(s(1), s(21))
    nc.vector.reciprocal(s(2), s(20))
    nc.vector.tensor_scalar_mul(s(3), s(1), -1.0)
    nc.vector.tensor_scalar(s(4), s(1), 8191.0, s(2), ALU.mult, ALU.add)
    nc.vector.tensor_scalar_mul(s(5), s(1), 1.0 / 32.0)
    nc.vector.tensor_scalar_mul(s(7), s(1), -0.5 / 1024.0)
    nc.vector.tensor_scalar_mul(s(9), s(1), 0.5 / 1024.0)
    nc.vector.tensor_mul(tm(0), s(5), s(5))
    nc.vector.tensor_scalar_mul(s(8), tm(0), 0.5)
    nc.scalar.activation(s(14), s(21), AF.Ln, scale=2.0 * math.pi)
    nc.scalar.activation(s(15), s(20), AF.Ln)
    nc.vector.tensor_scalar_mul(s(17), s(1), 8192.0)
    nc.vector.tensor_scalar(tm(2), s(1), 8192.0, s(2), ALU.mult, ALU.add)
    nc.vector.reciprocal(s(13), tm(2))
    nc.scalar.activation(s(16), s(13), AF.Ln)
    nc.vector.tensor_scalar_mul(s(18), s(1), 1.0 / 8192.0)
    # s19 = -4096*L2pinv + 0.5*(LpvW - Lpv);  s24 = s19 - LWC = 128*inv_nv
    nc.vector.tensor_sub(tm(5), s(16), s(15))
    nc.vector.tensor_scalar_mul(tm(5), tm(5), 0.5)
    nc.vector.tensor_scalar(tm(6), s(14), -4096.0, tm(5), ALU.mult, ALU.add)
    nc.vector.tensor_copy(s(19), tm(6))
    nc.vector.tensor_scalar_mul(s(24), s(1), 128.0)

    # ---------- per-candidate coefficient vectors [P, U] ----------
    gti = pers.tile([P, U], mybir.dt.int32)
    nc.gpsimd.iota(gti[:], [[1, U]], base=0, channel_multiplier=U)
    gt = pers.tile([P, U], F32)
    nc.vector.tensor_copy(gt[:], gti[:])

    nf = pers.tile([P, U], F32)
    nc.gpsimd.tensor_scalar_add(nf[:], gt[:], 1.0)
    zL = pers.tile([P, U], F32)
    nc.vector.tensor_scalar(zL[:], nf[:], sb(1), sb(2), ALU.mult, ALU.add)
    pvnL = pers.tile([P, U], F32)
    nc.vector.reciprocal(pvnL[:], zL[:])
    zR = pers.tile([P, U], F32)
    nc.vector.tensor_scalar(zR[:], gt[:], sb(3), sb(4), ALU.mult, ALU.add)
    pvnR = pers.tile([P, U], F32)
    nc.vector.reciprocal(pvnR[:], zR[:])
    lpvnL = pers.tile([P, U], F32)
    nc.scalar.activation(lpvnL[:], pvnL[:], AF.Ln)
    lpvnR = pers.tile([P, U], F32)
    nc.scalar.activation(lpvnR[:], pvnR[:], AF.Ln)
    kc2 = pers.tile([P, U], F32)
    nc.gpsimd.tensor_add(kc2[:], lpvnL[:], lpvnR[:])

    nRf = pers.tile([P, U], F32)
    nc.gpsimd.tensor_scalar(nRf[:], gt[:], -1.0, 8191.0, ALU.mult, ALU.add)
    gc = pers.tile([P, U], F32)
    nc.gpsimd.tensor_scalar_max(gc[:], gt[:], 1.0)
    inv_n1 = pers.tile([P, U], F32)
    nc.vector.reciprocal(inv_n1[:], gc[:])
    nR1c = pers.tile([P, U], F32)
    nc.gpsimd.tensor_scalar(nR1c[:], gt[:], -1.0, 8190.0, ALU.mult, ALU.add)
    nc.gpsimd.tensor_scalar_max(nR1c[:], nR1c[:], 1.0)
    inv_nR1 = pers.tile([P, U], F32)
    nc.vector.reciprocal(inv_nR1[:], nR1c[:])
    inv_n = pers.tile([P, U], F32)
    nc.vector.reciprocal(inv_n[:], nf[:])
    nRc = pers.tile([P, U], F32)
    nc.gpsimd.tensor_scalar_max(nRc[:], nRf[:], 1.0)
    inv_nR = pers.tile([P, U], F32)
    nc.vector.reciprocal(inv_nR[:], nRc[:])

    n_n1 = pers.tile([P, U], F32)
    nc.gpsimd.tensor_mul(n_n1[:], nf[:], inv_n1[:])
    nR_nR1 = pers.tile([P, U], F32)
    nc.gpsimd.tensor_mul(nR_nR1[:], nRf[:], inv_nR1[:])
    i_nn1 = pers.tile([P, U], F32)
    nc.gpsimd.tensor_mul(i_nn1[:], inv_n[:], inv_n1[:])
    i_nRnR1 = pers.tile([P, U], F32)
    nc.gpsimd.tensor_mul(i_nRnR1[:], inv_nR[:], inv_nR1[:])

    CA2L = pers.tile([P, U], F32)
    q1 = pers.tile([P, U], F32)
    nc.scalar.activation(q1[:], pvnL[:], AF.Copy, scale=sb(8))
    nc.vector.tensor_scalar(CA2L[:], i_nn1[:], sb(9), None, ALU.mult)
    nc.gpsimd.tensor_add(CA2L[:], CA2L[:], q1[:])
    CA2R = pers.tile([P, U], F32)
    q1b = pers.tile([P, U], F32)
    nc.scalar.activation(q1b[:], pvnR[:], AF.Copy, scale=sb(8))
    nc.vector.tensor_scalar(CA2R[:], i_nRnR1[:], sb(9), None, ALU.mult)
    nc.gpsimd.tensor_add(CA2R[:], CA2R[:], q1b[:])
    CBL = pers.tile([P, U], F32)
    nc.scalar.activation(CBL[:], n_n1[:], AF.Copy, scale=sb(7))
    CBR = pers.tile([P, U], F32)
    nc.scalar.activation(CBR[:], nR_nR1[:], AF.Copy, scale=sb(7))
    CBD = pers.tile([P, U], F32)
    nc.vector.tensor_sub(CBD[:], CBL[:], CBR[:])

    # Cc = 0.5*kc2 + (-4096*L2pinv - Lpv) - LWC + mask, with
    # LWC = s19 - 128*inv_nv  =>  const = 0.5*kc2 - Lpv - 0.5*(LpvW-Lpv)
    #                                  + 128*inv_nv
    Cc = pers.tile([P, U], F32)
    ccs = pers.tile([P, 1], F32)
    # ccs = -Lpv - 0.5*(LpvW - Lpv) + 128*inv_nv = -0.5*Lpv - 0.5*LpvW + s24
    nc.vector.tensor_add(tm(3), s(15), s(16))
    nc.vector.tensor_scalar(ccs[:], tm(3), -0.5, s(24), ALU.mult, ALU.add)
    nc.vector.tensor_scalar(Cc[:], kc2[:], 0.5, ccs[:], ALU.mult, ALU.add)
    mlo = pers.tile([P, U], F32)
    nc.vector.tensor_scalar(mlo[:], gt[:], 14.5, NEG, ALU.is_lt, ALU.mult)
    mhi = pers.tile([P, U], F32)
    nc.vector.tensor_scalar(mhi[:], gt[:], 8174.5, NEG, ALU.is_ge, ALU.mult)
    nc.gpsimd.tensor_add(Cc[:], Cc[:], mlo[:])
    nc.gpsimd.tensor_add(Cc[:], Cc[:], mhi[:])

    # partition masks for the near-end sum: a1 = (p > NE_P0), a2 = (p == NE_P0)
    pidx_i = pers.tile([P, 1], mybir.dt.int32)
    nc.gpsimd.iota(pidx_i[:], [[1, 1]], base=0, channel_multiplier=1)
    pidx = pers.tile([P, 1], F32)
    nc.vector.tensor_copy(pidx[:], pidx_i[:])
    a1 = pers.tile([P, 1], F32)
    nc.vector.tensor_scalar(a1[:], pidx[:], NE_P0 + 0.5, None, ALU.is_ge)
    a2 = pers.tile([P, 1], F32)
    nc.vector.tensor_scalar(a2[:], pidx[:], NE_P0 - 0.5, None, ALU.is_ge)
    nc.vector.tensor_sub(a2[:], a2[:], a1[:])

    zeros = pers.tile([P, max(CHUNKS_FAST) * U], F32)
    nc.gpsimd.memset(zeros[:], 0.0)

    # bund: [0:16) maxbf | [16:32) Zp | [32:48) En | [48:64) At | [64:80) Btot
    bund = pers.tile([P, 80], F32)

    # ---------- per-chunk pipeline ----------
    for ci, (bs, bc) in enumerate(chunks):
        xt = xts[ci]
        V = bc * U
        HU = U // 2

        # fold tree on DVE (fp16, 2x packed mode); fold1 split per u-half
        f1 = wk.tile([P, bc, U, 16], F16, tag="f1")
        for h in range(2):
            nc.vector.tensor_tensor(
                f1[:, :, h * HU : (h + 1) * HU, :],
                xt[:, :, h * HU : (h + 1) * HU, 0:16],
                xt[:, :, h * HU : (h + 1) * HU, 16:32],
                ALU.add,
            )
        f2 = wk.tile([P, bc, U, 8], F16, tag="f2")
        nc.vector.tensor_tensor(f2[:], f1[:, :, :, 0:8], f1[:, :, :, 8:16], ALU.add)
        f3 = wk.tile([P, bc, U, 4], F16, tag="f3")
        nc.vector.tensor_tensor(f3[:], f2[:, :, :, 0:4], f2[:, :, :, 4:8], ALU.add)
        f4 = wk.tile([P, bc, U, 2], F16, tag="f4")
        nc.vector.tensor_tensor(f4[:], f3[:, :, :, 0:2], f3[:, :, :, 2:4], ALU.add)

        # sr (for sq) on Pool; prefix sums via DVE scans (fp32 state)
        sr = wk.tile([P, bc, U], F16, tag="sr")
        nc.gpsimd.tensor_tensor(sr[:], f4[:, :, :, 0], f4[:, :, :, 1], ALU.add)
        sq = wk.tile([P, bc, U], F32, tag="sq")
        nc.scalar.activation(sq[:], sr[:], AF.Square)

        A = wk.tile([P, bc, U], F32, tag="A")
        nc.vector.tensor_tensor_scan(
            A[:].rearrange("p b u -> p (b u)"),
            f4[:, :, :, 0].rearrange("p b u -> p (b u)"),
            f4[:, :, :, 1].rearrange("p b u -> p (b u)"),
            0.0,
            ALU.add,
            ALU.add,
        )
        Bt = wk.tile([P, bc, U], F32, tag="Bt")
        nc.vector.tensor_tensor_scan(
            Bt[:].rearrange("p b u -> p (b u)"),
            sq[:].rearrange("p b u -> p (b u)"),
            zeros[:, 0:V],
            0.0,
            ALU.add,
            ALU.add,
        )

        # carry fix: row-prev, per-row totals, triangular matmul on PE
        rv = wk.tile([P, 2 * bc], F32, tag="rv")
        nc.gpsimd.memset(rv[:, 0:1], 0.0)
        nc.gpsimd.memset(rv[:, bc : bc + 1], 0.0)
        nc.vector.tensor_copy(rv[:, 1:bc], A[:, 0 : bc - 1, U - 1])
        nc.vector.tensor_copy(rv[:, bc + 1 : 2 * bc], Bt[:, 0 : bc - 1, U - 1])
        ct = wk.tile([P, 2 * bc], F32, tag="ct")
        nc.vector.tensor_sub(ct[:, 0:bc], A[:, :, U - 1], rv[:, 0:bc])
        nc.vector.tensor_sub(ct[:, bc : 2 * bc], Bt[:, :, U - 1], rv[:, bc : 2 * bc])
        g_ps = psp.tile([P, 2 * bc], F32)
        nc.tensor.matmul(g_ps[:], ut_t[:], ct[:])
        tot_ps = psp.tile([P, 2 * bc], F32)
        nc.tensor.matmul(tot_ps[:], ones_t[:], ct[:])

        # At/Btot -> bund (SBUF staging; Pool can't read PSUM)
        nc.scalar.copy(
            bund[:, 48:80].rearrange("p (s c) -> p s c", s=2)[:, :, bs : bs + bc],
            tot_ps[:].rearrange("p (s c) -> p s c", s=2),
        )

        # per-row offsets (PSUM reads on DVE)
        offA = wk.tile([P, bc], F32, tag="offA")
        nc.vector.tensor_sub(offA[:], g_ps[:, 0:bc], rv[:, 0:bc])
        noffB = wk.tile([P, bc], F32, tag="noffB")
        nc.vector.tensor_sub(noffB[:], rv[:, bc : 2 * bc], g_ps[:, bc : 2 * bc])
        Sp = wk.tile([P, bc], F32, tag="Sp")
        nc.vector.tensor_sub(Sp[:], tot_ps[:, 0:bc], offA[:])

        # A-side: AR_b = S'_b - A_b (Pool), A2_b = (A_b + offA_b)^2 (Act),
        # AR2 chunk-wide (Act)
        AR = wk.tile([P, bc, U], F32, tag="AR")
        A2 = wk.tile([P, bc, U], F32, tag="A2")
        for b in range(bc):
            nc.gpsimd.tensor_scalar(
                AR[:, b], A[:, b], -1.0, Sp[:, b : b + 1], ALU.mult, ALU.add
            )
            nc.scalar.activation(
                A2[:, b], A[:, b], AF.Square, bias=offA[:, b : b + 1]
            )
        AR2 = wk.tile([P, bc, U], F32, tag="AR2")
        nc.scalar.activation(AR2[:], AR[:], AF.Square)

        # B-side fused per row (DVE STT): m = (Bt - noffB)*CBD ; s2 = CBR*Q + m
        s2t = wk.tile([P, bc, U], F32, tag="s2t")
        mt = wk.tile([P, bc, U], F32, tag="mt")
        for b in range(bc):
            nc.vector.scalar_tensor_tensor(
                mt[:, b], Bt[:, b], noffB[:, b : b + 1], CBD[:],
                ALU.subtract, ALU.mult,
            )
            nc.vector.scalar_tensor_tensor(
                s2t[:, b], CBR[:], bund[:, 64 + bs + b : 65 + bs + b], mt[:, b],
                ALU.mult, ALU.add,
            )

        # p2 = A2*CA2L, p5 = AR2*CA2R, s1 = p2+p5, s3 = s1+s2, bf = s3+Cc (Pool)
        def cb(t):
            return t[:].unsqueeze(1).broadcast_to([P, bc, U])

        p2 = wk.tile([P, bc, U], F32, tag="p2")
        nc.gpsimd.tensor_mul(p2[:], A2[:], cb(CA2L))
        p5 = wk.tile([P, bc, U], F32, tag="p5")
        nc.gpsimd.tensor_mul(p5[:], AR2[:], cb(CA2R))
        s1t = wk.tile([P, bc, U], F32, tag="s1t")
        nc.gpsimd.tensor_add(s1t[:], p2[:], p5[:])
        s3t = wk.tile([P, bc, U], F32, tag="s3t")
        nc.gpsimd.tensor_add(s3t[:], s1t[:], s2t[:])
        bf = wk.tile([P, bc, U], F32, tag="bf")
        nc.gpsimd.tensor_add(bf[:], s3t[:], cb(Cc))

        # maxbf (DVE), e + Zp per row (Act accum), near-end suffix (DVE)
        nc.vector.tensor_reduce(bund[:, bs : bs + bc], bf[:], AX.X, ALU.max)
        e = wk.tile([P, bc, U], F32, tag="e")
        for b in range(bc):
            nc.scalar.activation(
                e[:, b], bf[:, b], AF.Exp,
                accum_out=bund[:, 16 + bs + b : 17 + bs + b],
            )
        ssuf = wk.tile([P, bc], F32, tag="ssuf")
        nc.vector.tensor_reduce(ssuf[:], e[:, :, NE_U0:U], AX.X, ALU.add)
        t2 = wk.tile([P, bc], F32, tag="t2")
        nc.vector.tensor_scalar(t2[:], ssuf[:], a2[:], None, ALU.mult)
        nc.vector.scalar_tensor_tensor(
            bund[:, 32 + bs : 32 + bs + bc],
            bund[:, 16 + bs : 16 + bs + bc], a1[:], t2[:],
            ALU.mult, ALU.add,
        )

    # ---------- finale ----------
    Tall = ps1.tile([BL, 5, P], F32)
    nc.tensor.transpose(Tall[:, 0, :], bund[:, 0:BL], id_t[:])
    nc.tensor.transpose(Tall[:, 1, :], bund[:, 16 : 16 + BL], id_t[:])
    nc.tensor.transpose(Tall[:, 2, :], bund[:, 32 : 32 + BL], id_t[:])
    nc.tensor.transpose(Tall[:, 3, :], bund[:, 48 : 48 + BL], id_t[:])
    nc.tensor.transpose(Tall[:, 4, :], bund[:, 64 : 64 + BL], id_t[:])
    M16 = pers.tile([BL, 1], F32)
    nc.vector.tensor_reduce(M16[:], Tall[:, 0, :], AX.X, ALU.max)
    Z16 = pers.tile([BL, 1], F32)
    nc.vector.tensor_reduce(Z16[:], Tall[:, 1, :], AX.X, ALU.add)
    E16 = pers.tile([BL, 1], F32)
    nc.vector.tensor_reduce(E16[:], Tall[:, 2, :], AX.X, ALU.add)
    At16 = Tall[:, 3, 0:1]
    Bt16 = Tall[:, 4, 0:1]

    # rebfW = -0.5*(term1+term2-term3) + 128*inv_nv
    t1b = pers.tile([BL, 1], F32)
    nc.scalar.activation(t1b[:], At16, AF.Square, scale=1.0 / 32.0)
    v2 = pers.tile([BL, 1], F32)
    nc.vector.tensor_scalar_mul(v2[:], t1b[:], 1.0 / 8192.0)
    vW = pers.tile([BL, 1], F32)
    nc.vector.scalar_tensor_tensor(
        vW[:], Bt16, 1.0 / 1024.0, v2[:], ALU.mult, ALU.subtract
    )
    nc.vector.tensor_scalar(vW[:], vW[:], 1.0 / 8191.0, 1.0e-8, ALU.mult, ALU.max)
    term1 = pers.tile([BL, 1], F32)
    nc.vector.tensor_scalar(term1[:], vW[:], sb(17, BL), None, ALU.mult)
    term2 = pers.tile([BL, 1], F32)
    nc.vector.tensor_scalar(term2[:], t1b[:], sb(18, BL), None, ALU.mult)
    uu = pers.tile([BL, 1], F32)
    nc.vector.tensor_scalar(uu[:], At16, sb(5, BL), None, ALU.mult)
    u2 = pers.tile([BL, 1], F32)
    nc.vector.tensor_mul(u2[:], uu[:], uu[:])
    term3 = pers.tile([BL, 1], F32)
    nc.vector.tensor_scalar(term3[:], u2[:], sb(13, BL), None, ALU.mult)
    tsum = pers.tile([BL, 1], F32)
    nc.vector.tensor_add(tsum[:], term1[:], term2[:])
    nc.vector.tensor_sub(tsum[:], tsum[:], term3[:])
    rebfW = pers.tile([BL, 1], F32)
    nc.vector.tensor_scalar(rebfW[:], tsum[:], -0.5, sb(24, BL), ALU.mult, ALU.add)

    sigin = pers.tile([BL, 1], F32)
    nc.vector.tensor_sub(sigin[:], M16[:], rebfW[:])
    sig = pers.tile([BL, 1], F32)
    nc.scalar.activation(sig[:], sigin[:], AF.Sigmoid)
    invZ = pers.tile([BL, 1], F32)
    nc.vector.reciprocal(invZ[:], Z16[:])
    ratio = pers.tile([BL, 1], F32)
    nc.vector.tensor_mul(ratio[:], E16[:], invZ[:])
    outv = pers.tile([BL, 1], F32)
    nc.vector.tensor_mul(outv[:], sig[:], ratio[:])
    nc.sync.dma_start(out[:], outv[:])


# ======================================================================
# general-pm fallback: the original f32 kernel (unchanged numerics)
# ======================================================================

BC = 4
NCHUNK = BL // BC
UF_SCHED = [16, 32, 32, 32]
CHUNK_SIZES = [4, 4, 4, 4]
XP_BUFS = 2
WK_BUFS = 3
LO_INV_U = 15
HI_INV_U = 8174 - 127 * U + 1


def build_body(ctx, tc, x, params, utc, idc, out, pm_zero=False):
    nc = tc.nc
    pers = ctx.enter_context(tc.tile_pool(name="pers", bufs=1))
    xp = ctx.enter_context(tc.tile_pool(name="xp", bufs=XP_BUFS))
    wk = ctx.enter_context(tc.tile_pool(name="wk", bufs=WK_BUFS))
    psp = ctx.enter_context(tc.tile_pool(name="psp", bufs=2, space="PSUM"))
    ps1 = ctx.enter_context(tc.tile_pool(name="ps1", bufs=1, space="PSUM"))

    ut_t = pers.tile([P, P], F32)
    ones_t = pers.tile([P, P], F32)
    id_t = pers.tile([P, P], F32)
    gt = pers.tile([P, U], F32)
    ptile = pers.tile([P, 3], F32)
    nc.sync.dma_start(ptile[:], params[:])
    nc.gpsimd.memset(ones_t[:], 1.0)
    gti = pers.tile([P, U], mybir.dt.int32)
    nc.gpsimd.iota(gti[:], [[1, U]], base=0, channel_multiplier=U)
    nc.vector.tensor_copy(gt[:], gti[:])

    chunks = []
    o = 0
    for c in CHUNK_SIZES:
        chunks.append((o, c))
        o += c
    assert o == BL
    xts = []
    for ci, (bs, bc) in enumerate(chunks):
        xt = xp.tile([P, bc, U, N], F32, tag="xt")
        src = x[bs : bs + bc].rearrange("b (p u) n -> p b u n", p=P)
        if ci == 0:
            QU = U // 4
            for q in range(4):
                nc.sync.dma_start(
                    xt[:, :, q * QU : (q + 1) * QU, :],
                    src[:, :, q * QU : (q + 1) * QU, :],
                )
        else:
            HU = U // 2
            nc.sync.dma_start(xt[:, :, 0:HU, :], src[:, :, 0:HU, :])
            nc.sync.dma_start(xt[:, :, HU:U, :], src[:, :, HU:U, :])
        if ci == 0:
            nc.sync.dma_start(ut_t[:], utc[:])
            nc.sync.dma_start(id_t[:], idc[:])
        xts.append(xt)

    sv = pers.tile([P, NS], F32)
    tmp = pers.tile([P, 8], F32)

    def s(i):
        return sv[:, i : i + 1]

    def tm(i):
        return tmp[:, i : i + 1]

    nc.scalar.activation(tm(0), ptile[:, 1:2], AF.Exp)
    nc.vector.tensor_scalar_add(tm(0), tm(0), 1.0)
    nc.scalar.activation(s(20), tm(0), AF.Ln)
    nc.scalar.activation(tm(1), ptile[:, 2:3], AF.Exp)
    nc.vector.tensor_scalar_add(tm(1), tm(1), 1.0)
    nc.scalar.activation(s(21), tm(1), AF.Ln)
    nc.vector.tensor_copy(s(0), ptile[:, 0:1])
    nc.vector.reciprocal(s(1), s(21))
    nc.vector.reciprocal(s(2), s(20))
    nc.vector.tensor_scalar_mul(s(3), s(1), -1.0)
    nc.vector.tensor_scalar(s(4), s(1), 8191.0, s(2), ALU.mult, ALU.add)
    nc.vector.tensor_scalar_mul(s(5), s(1), 1.0 / 32.0)
    nc.vector.tensor_mul(s(6), s(0), s(2))
    nc.vector.tensor_scalar_mul(s(7), s(1), -0.5 / 1024.0)
    nc.vector.tensor_scalar_mul(s(9), s(1), 0.5 / 1024.0)
    nc.vector.tensor_mul(tm(0), s(5), s(5))
    nc.vector.tensor_scalar_mul(s(8), tm(0), 0.5)
    nc.vector.tensor_mul(s(10), s(6), s(5))
    nc.vector.tensor_mul(tm(1), s(6), s(6))
    nc.vector.tensor_scalar_mul(s(11), tm(1), 0.5)
    nc.scalar.activation(s(14), s(21), AF.Ln, scale=2.0 * math.pi)
    nc.scalar.activation(s(15), s(20), AF.Ln)
    nc.vector.tensor_scalar_mul(s(17), s(1), 8192.0)
    nc.vector.tensor_scalar(tm(2), s(1), 8192.0, s(2), ALU.mult, ALU.add)
    nc.vector.reciprocal(s(13), tm(2))
    nc.scalar.activation(s(16), s(13), AF.Ln)
    nc.vector.tensor_scalar_mul(s(18), s(1), 1.0 / 8192.0)
    nc.vector.tensor_mul(tm(3), s(0), s(0))
    nc.vector.tensor_mul(s(22), tm(3), s(2))
    nc.vector.tensor_scalar_mul(s(23), s(14), -4096.0)
    nc.vector.tensor_sub(tm(4), s(23), s(15))
    nc.vector.tensor_sub(s(12), tm(4), s(22))
    nc.vector.tensor_sub(tm(5), s(16), s(15))
    nc.vector.tensor_scalar_mul(tm(5), tm(5), 0.5)
    nc.vector.tensor_add(tm(6), s(23), tm(5))
    nc.vector.tensor_scalar_mul(tm(7), s(22), -0.5)
    nc.vector.tensor_add(s(19), tm(6), tm(7))

    def sb(i, np_=P, p0=0):
        return sv[p0 : p0 + np_, i : i + 1]

    nf = pers.tile([P, U], F32)
    nc.vector.tensor_scalar_add(nf[:], gt[:], 1.0)
    zL = pers.tile([P, U], F32)
    nc.vector.tensor_scalar(zL[:], nf[:], sb(1), sb(2), ALU.mult, ALU.add)
    pvnL = pers.tile([P, U], F32)
    nc.vector.reciprocal(pvnL[:], zL[:])
    zR = pers.tile([P, U], F32)
    nc.vector.tensor_scalar(zR[:], gt[:], sb(3), sb(4), ALU.mult, ALU.add)
    pvnR = pers.tile([P, U], F32)
    nc.vector.reciprocal(pvnR[:], zR[:])
    lpvnL = pers.tile([P, U], F32)
    nc.scalar.activation(lpvnL[:], pvnL[:], AF.Ln)
    lpvnR = pers.tile([P, U], F32)
    nc.scalar.activation(lpvnR[:], pvnR[:], AF.Ln)
    kc2 = pers.tile([P, U], F32)
    nc.vector.tensor_add(kc2[:], lpvnL[:], lpvnR[:])

    nRf = pers.tile([P, U], F32)
    nc.vector.tensor_scalar(nRf[:], gt[:], -1.0, 8191.0, ALU.mult, ALU.add)
    gc = pers.tile([P, U], F32)
    nc.vector.tensor_scalar_max(gc[:], gt[:], 1.0)
    inv_n1 = pers.tile([P, U], F32)
    nc.vector.reciprocal(inv_n1[:], gc[:])
    nR1c = pers.tile([P, U], F32)
    nc.vector.tensor_scalar(nR1c[:], gt[:], -1.0, 8190.0, ALU.mult, ALU.add)
    nc.vector.tensor_scalar_max(nR1c[:], nR1c[:], 1.0)
    inv_nR1 = pers.tile([P, U], F32)
    nc.vector.reciprocal(inv_nR1[:], nR1c[:])
    inv_n = pers.tile([P, U], F32)
    nc.vector.reciprocal(inv_n[:], nf[:])
    inv_nR = pers.tile([P, U], F32)
    nRc = pers.tile([P, U], F32)
    nc.vector.tensor_scalar_max(nRc[:], nRf[:], 1.0)
    nc.vector.reciprocal(inv_nR[:], nRc[:])

    n_n1 = pers.tile([P, U], F32)
    nc.vector.tensor_mul(n_n1[:], nf[:], inv_n1[:])
    nR_nR1 = pers.tile([P, U], F32)
    nc.vector.tensor_mul(nR_nR1[:], nRf[:], inv_nR1[:])
    i_nn1 = pers.tile([P, U], F32)
    nc.vector.tensor_mul(i_nn1[:], inv_n[:], inv_n1[:])
    i_nRnR1 = pers.tile([P, U], F32)
    nc.vector.tensor_mul(i_nRnR1[:], inv_nR[:], inv_nR1[:])

    CBL = pers.tile([P, U], F32)
    nc.scalar.activation(CBL[:], n_n1[:], AF.Copy, scale=sb(7))
    CBR = pers.tile([P, U], F32)
    nc.scalar.activation(CBR[:], nR_nR1[:], AF.Copy, scale=sb(7))
    CA2L = pers.tile([P, U], F32)
    q1 = pers.tile([P, U], F32)
    nc.scalar.activation(q1[:], pvnL[:], AF.Copy, scale=sb(8))
    q2 = pers.tile([P, U], F32)
    nc.scalar.activation(q2[:], i_nn1[:], AF.Copy, scale=sb(9))
    nc.vector.tensor_add(CA2L[:], q1[:], q2[:])
    CA2R = pers.tile([P, U], F32)
    q1b = pers.tile([P, U], F32)
    nc.scalar.activation(q1b[:], pvnR[:], AF.Copy, scale=sb(8))
    q2b = pers.tile([P, U], F32)
    nc.scalar.activation(q2b[:], i_nRnR1[:], AF.Copy, scale=sb(9))
    nc.vector.tensor_add(CA2R[:], q1b[:], q2b[:])
    CAL = pers.tile([P, U], F32)
    nc.scalar.activation(CAL[:], pvnL[:], AF.Copy, scale=sb(10))
    CAR = pers.tile([P, U], F32)
    nc.scalar.activation(CAR[:], pvnR[:], AF.Copy, scale=sb(10))
    Cc = pers.tile([P, U], F32)
    p12 = pers.tile([P, U], F32)
    nc.vector.tensor_add(p12[:], pvnL[:], pvnR[:])
    cc1 = pers.tile([P, U], F32)
    nc.scalar.activation(cc1[:], p12[:], AF.Copy, scale=sb(11))
    cct = pers.tile([P, U], F32)
    nc.vector.tensor_scalar(cct[:], kc2[:], 0.5, sb(12), ALU.mult, ALU.add)
    nc.vector.tensor_add(Cc[:], cc1[:], cct[:])
    mlo = pers.tile([P, U], F32)
    nc.vector.tensor_scalar(mlo[:], gt[:], 14.5, NEG, ALU.is_lt, ALU.mult)
    mhi = pers.tile([P, U], F32)
    nc.vector.tensor_scalar(mhi[:], gt[:], 8174.5, NEG, ALU.is_ge, ALU.mult)
    nc.vector.tensor_add(Cc[:], Cc[:], mlo[:])
    nc.vector.tensor_add(Cc[:], Cc[:], mhi[:])
    nemask = pers.tile([P, U], F32)
    nc.vector.tensor_scalar(nemask[:], gt[:], 6552.5, None, ALU.is_ge)

    bund = pers.tile([P, 80], F32)
    zeros = pers.tile([P, max(CHUNK_SIZES) * U], F32)
    nc.gpsimd.memset(zeros[:], 0.0)

    Tall = ps1.tile([BL, 5, P], F32)
    Tm = Tall[:, 0, :]
    Tz = Tall[:, 1, :]
    Te = Tall[:, 2, :]
    Ta = Tall[:, 3, :]
    Tb = Tall[:, 4, :]
    M16 = pers.tile([BL, 1], F32)
    d = pers.tile([BL, P], F32)
    w = pers.tile([BL, P], F32)
    bfW = pers.tile([BL, 1], F32)
    sig = pers.tile([BL, 1], F32)

    def trace_bfw():
        nc.tensor.transpose(Ta, bund[:, 48 : 48 + BL], id_t[:])
        nc.tensor.transpose(Tb, bund[:, 64 : 64 + BL], id_t[:])
        At16 = Ta[:, 0:1]
        Bt16 = Tb[:, 0:1]
        t1 = pers.tile([BL, 1], F32)
        nc.scalar.activation(t1[:], At16, AF.Square, scale=1.0 / 32.0)
        v2 = pers.tile([BL, 1], F32)
        nc.vector.tensor_scalar_mul(v2[:], t1[:], 1.0 / 8192.0)
        vW = pers.tile([BL, 1], F32)
        nc.vector.scalar_tensor_tensor(
            vW[:], Bt16, 1.0 / 1024.0, v2[:], ALU.mult, ALU.subtract
        )
        nc.vector.tensor_scalar(vW[:], vW[:], 1.0 / 8191.0, 1.0e-8, ALU.mult, ALU.max)
        term1 = pers.tile([BL, 1], F32)
        nc.vector.tensor_scalar_mul(term1[:], vW[:], sb(17, BL))
        term2 = pers.tile([BL, 1], F32)
        nc.vector.tensor_scalar_mul(term2[:], t1[:], sb(18, BL))
        uu = pers.tile([BL, 1], F32)
        nc.scalar.activation(uu[:], At16, AF.Identity, bias=sb(6, BL), scale=sb(5, BL))
        u2 = pers.tile([BL, 1], F32)
        nc.scalar.activation(u2[:], uu[:], AF.Square)
        term3 = pers.tile([BL, 1], F32)
        nc.vector.tensor_scalar_mul(term3[:], u2[:], sb(13, BL))
        tsum = pers.tile([BL, 1], F32)
        nc.vector.tensor_add(tsum[:], term1[:], term2[:])
        nc.vector.tensor_sub(tsum[:], tsum[:], term3[:])
        nc.vector.tensor_scalar(bfW[:], tsum[:], -0.5, sb(19, BL), ALU.mult, ALU.add)

    def trace_maxw():
        nc.tensor.transpose(Tm, bund[:, 0:BL], id_t[:])
        nc.vector.tensor_reduce(M16[:], Tm, AX.X, ALU.min, negate=True)
        nc.vector.tensor_scalar(d[:], Tm, -1.0, M16[:], ALU.mult, ALU.subtract)
        nc.scalar.activation(w[:], d[:], AF.Exp)
        sigin = pers.tile([BL, 1], F32)
        nc.vector.tensor_sub(sigin[:], M16[:], bfW[:])
        nc.scalar.activation(sig[:], sigin[:], AF.Sigmoid)

    xhs = [None] * len(chunks)

    def ufof(ci):
        return UF_SCHED[ci] if UF_SCHED is not None else 32

    def trace_fold(ci):
        uf = ufof(ci)
        bc = chunks[ci][1]
        if uf > 0:
            xh = wk.tile([P, bc, uf, 16], F32, tag="xh")
            nc.gpsimd.tensor_add(
                xh[:], xts[ci][:, :, 0:uf, 0:16], xts[ci][:, :, 0:uf, 16:32]
            )
            xhs[ci] = xh

    trace_fold(0)
    for ci, (bs, bc) in enumerate(chunks):
        xt = xts[ci]
        last = ci == len(chunks) - 1
        if not last:
            trace_fold(ci + 1)

        uf = ufof(ci)
        sr = wk.tile([P, bc, U], F32)
        if uf > 0:
            if ci == 0 and uf == 16:
                for q in range(1, 4):
                    nc.vector.tensor_reduce(
                        sr[:, :, q * 16 : (q + 1) * 16],
                        xt[:, :, q * 16 : (q + 1) * 16, :],
                        AX.X,
                        ALU.add,
                    )
            elif uf < U:
                nc.vector.tensor_reduce(
                    sr[:, :, uf:U], xt[:, :, uf:U, :], AX.X, ALU.add
                )
            nc.vector.tensor_reduce(sr[:, :, 0:uf], xhs[ci][:], AX.X, ALU.add)
        else:
            HU = U // 2
            nc.vector.tensor_reduce(
                sr[:, :, 0:HU], xt[:, :, 0:HU, :], AX.X, ALU.add
            )
            nc.vector.tensor_reduce(
                sr[:, :, HU:U], xt[:, :, HU:U, :], AX.X, ALU.add
            )
        sq = wk.tile([P, bc, U], F32)
        nc.scalar.activation(sq[:], sr[:], AF.Square)

        A = wk.tile([P, bc, U], F32)
        nc.vector.tensor_tensor_scan(
            A[:].rearrange("p b u -> p (b u)"),
            sr[:].rearrange("p b u -> p (b u)"),
            zeros[:, 0 : bc * U],
            0.0,
            ALU.add,
            ALU.add,
        )
        Bt_ = wk.tile([P, bc, U], F32)
        nc.vector.tensor_tensor_scan(
            Bt_[:].rearrange("p b u -> p (b u)"),
            sq[:].rearrange("p b u -> p (b u)"),
            zeros[:, 0 : bc * U],
            0.0,
            ALU.add,
            ALU.add,
        )

        rv = wk.tile([P, 2 * bc], F32)
        nc.gpsimd.memset(rv[:, 0:1], 0.0)
        nc.gpsimd.memset(rv[:, bc : bc + 1], 0.0)
        nc.vector.tensor_copy(rv[:, 1:bc], A[:, 0 : bc - 1, U - 1])
        nc.vector.tensor_copy(rv[:, bc + 1 : 2 * bc], Bt_[:, 0 : bc - 1, U - 1])
        ct = wk.tile([P, 2 * bc], F32)
        nc.vector.tensor_sub(ct[:, 0:bc], A[:, :, U - 1], rv[:, 0:bc])
        nc.vector.tensor_sub(ct[:, bc : 2 * bc], Bt_[:, :, U - 1], rv[:, bc : 2 * bc])
        g_ps = psp.tile([P, 2 * bc], F32)
        nc.tensor.matmul(g_ps[:], ut_t[:], ct[:])
        tot_ps = psp.tile([P, 2 * bc], F32)
        nc.tensor.matmul(tot_ps[:], ones_t[:], ct[:])
        off = wk.tile([P, 2 * bc], F32)
        nc.vector.tensor_sub(off[:], g_ps[:], rv[:])

        offA_b = off[:, 0:bc].unsqueeze(2).broadcast_to([P, bc, U])
        offB_b = off[:, bc : 2 * bc].unsqueeze(2).broadcast_to([P, bc, U])
        nc.vector.tensor_add(A[:], A[:], offA_b)
        nc.vector.tensor_add(Bt_[:], Bt_[:], offB_b)

        nc.scalar.copy(bund[:, 48 + bs : 48 + bs + bc], tot_ps[:, 0:bc])
        nc.scalar.copy(bund[:, 64 + bs : 64 + bs + bc], tot_ps[:, bc : 2 * bc])
        if last:
            trace_bfw()
        At_b = (
            bund[:, 48 + bs : 48 + bs + bc].unsqueeze(2).broadcast_to([P, bc, U])
        )
        Btot_b = (
            bund[:, 64 + bs : 64 + bs + bc].unsqueeze(2).broadcast_to([P, bc, U])
        )
        AR = wk.tile([P, bc, U], F32)
        nc.gpsimd.tensor_sub(AR[:], At_b, A[:])
        BR = wk.tile([P, bc, U], F32)
        nc.gpsimd.tensor_sub(BR[:], Btot_b, Bt_[:])

        A2 = wk.tile([P, bc, U], F32)
        nc.scalar.activation(A2[:], A[:], AF.Square)
        AR2 = wk.tile([P, bc, U], F32)
        nc.scalar.activation(AR2[:], AR[:], AF.Square)

        def cb(t):
            return t[:].unsqueeze(1).broadcast_to([P, bc, U])

        bf = wk.tile([P, bc, U], F32)
        p2 = wk.tile([P, bc, U], F32)
        p3 = wk.tile([P, bc, U], F32)
        p5 = wk.tile([P, bc, U], F32)
        p6 = wk.tile([P, bc, U], F32)
        nc.vector.tensor_mul(p2[:], A2[:], cb(CA2L))
        nc.vector.tensor_mul(p3[:], Bt_[:], cb(CBL))
        nc.vector.tensor_mul(p5[:], AR2[:], cb(CA2R))
        (nc.vector if last else nc.gpsimd).tensor_mul(p6[:], BR[:], cb(CBR))
        if pm_zero:
            eng1 = nc.vector if last else nc.gpsimd
            eng1.tensor_add(p2[:], p2[:], p3[:])
            nc.vector.tensor_add(p5[:], p5[:], p6[:])
            eng1.tensor_add(p2[:], p2[:], cb(Cc))
            nc.vector.tensor_add(bf[:], p2[:], p5[:])
        else:
            p1 = wk.tile([P, bc, U], F32)
            p4 = wk.tile([P, bc, U], F32)
            nc.vector.tensor_mul(p1[:], A[:], cb(CAL))
            nc.vector.tensor_mul(p4[:], AR[:], cb(CAR))
            nc.gpsimd.tensor_add(p1[:], p1[:], p2[:])
            nc.vector.tensor_add(p3[:], p3[:], p4[:])
            nc.gpsimd.tensor_add(p5[:], p5[:], p6[:])
            nc.gpsimd.tensor_add(p1[:], p1[:], cb(Cc))
            nc.vector.tensor_add(p3[:], p3[:], p5[:])
            nc.vector.tensor_add(bf[:], p1[:], p3[:])

        nc.vector.tensor_reduce(
            bund[:, bs : bs + bc], bf[:], AX.X, ALU.max, negate=True
        )
        if last:
            trace_maxw()
        e = wk.tile([P, bc, U], F32)
        for b in range(bc):
            nc.scalar.activation(
                e[:, b, :],
                bf[:, b, :],
                AF.Exp,
                bias=bund[:, bs + b : bs + b + 1],
                accum_out=bund[:, 16 + bs + b : 17 + bs + b],
            )
        en = wk.tile([P, bc, U], F32)
        (nc.vector if last else nc.gpsimd).tensor_mul(en[:], e[:], cb(nemask))
        nc.vector.tensor_reduce(
            bund[:, 32 + bs : 32 + bs + bc], en[:], AX.X, ALU.add
        )

    nc.tensor.transpose(Tz, bund[:, 16 : 16 + BL], id_t[:])
    nc.tensor.transpose(Te, bund[:, 32 : 32 + BL], id_t[:])
    wz = pers.tile([BL, P], F32)
    Zb = pers.tile([BL, 1], F32)
    nc.vector.scalar_tensor_tensor(
        wz[:], w[:], 1.0, Tz, ALU.mult, ALU.mult, accum_out=Zb[:]
    )
    wn = pers.tile([BL, P], F32)
    Nb = pers.tile([BL, 1], F32)
    nc.vector.scalar_tensor_tensor(
        wn[:], w[:], 1.0, Te, ALU.mult, ALU.mult, accum_out=Nb[:]
    )
    invZ = pers.tile([BL, 1], F32)
    nc.vector.reciprocal(invZ[:], Zb[:])
    ratio = pers.tile([BL, 1], F32)
    nc.vector.tensor_mul(ratio[:], Nb[:], invZ[:])
    outv = pers.tile([BL, 1], F32)
    nc.vector.tensor_mul(outv[:], sig[:], ratio[:])
    nc.sync.dma_start(out[:], outv[:])


def host_consts():
    ut = np.triu(np.ones((P, P), np.float32), 1)
    ident = np.eye(P, dtype=np.float32)
    return ut, ident


def split_multi_waits(nc):
    """Walrus in this container allows one sync wait per instruction; move
    extra waits onto same-engine NOPs inserted immediately before."""
    import bass_rust

    nid = [0]
    for f in nc.m.functions:
        for b in f.blocks:
            insts = b.instructions
            i = 0
            while i < len(insts):
                ins = insts[i]
                si = ins.sync_info
                if si is not None and si.on_wait is not None and len(si.on_wait) > 1:
                    waits = list(si.on_wait)
                    for w in waits[:-1]:
                        nop = mybir.InstNoOp(
                            name=f"I-waitsplit-{nid[0]}", ins=[], outs=[]
                        )
                        nid[0] += 1
                        nop.engine = ins.engine
                        nop.sync_info = bass_rust.SyncInfo(
                            on_wait=[w], on_update=[]
                        )
                        insts.insert(i, nop)
                        i += 1
                    si.on_wait = waits[-1:]
                i += 1


_NC_CACHE = {}


def build_nc(split=True, reps=1, pm_zero=True):
    global _NC_CACHE
    key = (split, reps, pm_zero)
    if key in _NC_CACHE:
        return _NC_CACHE[key]
    nc = bass.Bass()
    xdt = F16 if pm_zero else F32
    x = nc.declare_dram_parameter("x", [BL, T, N], xdt, isOutput=False)
    params = nc.declare_dram_parameter("params", [P, 3], F32, isOutput=False)
    utc = nc.declare_dram_parameter("utc", [P, P], F32, isOutput=False)
    idc = nc.declare_dram_parameter("idc", [P, P], F32, isOutput=False)
    out = nc.declare_dram_parameter("out", [BL, 1], F32, isOutput=True)
    with nc.allow_low_precision(reason="fp16 x staging; fp32 accum state"):
        with tile.TileContext(nc) as tc:
            for _ in range(reps):
                with ExitStack() as ctx:
                    if pm_zero:
                        build_body_fast(
                            ctx, tc, x[:], params[:], utc[:], idc[:], out[:]
                        )
                    else:
                        build_body(
                            ctx, tc, x[:], params[:], utc[:], idc[:],
                            out[:], pm_zero=False,
                        )
    if split:
        split_multi_waits(nc)
    _NC_CACHE[key] = nc
    return nc


def make_in_maps(x, prior_mean, prior_var, noise_var, pm_zero=True):
    xdt = np.float16 if pm_zero else np.float32
    x = np.ascontiguousarray(np.asarray(x).astype(xdt))
    params = np.tile(
        np.array(
            [[float(prior_mean[0]), float(prior_var[0]), float(noise_var[0])]],
            dtype=np.float32,
        ),
        (P, 1),
    )
    ut, ident = host_consts()
    in_maps = []
    for c in range(NCORES):
        in_maps.append(
            {
                "x": x[c * BL : (c + 1) * BL],
                "params": params,
                "utc": ut,
                "idc": ident,
            }
        )
    return in_maps


def kernel(x, prior_mean, prior_var, noise_var):
    from concourse.bass_utils import run_bass_kernel_spmd

    pm_zero = float(np.asarray(prior_mean).reshape(-1)[0]) == 0.0
    in_maps = make_in_maps(x, prior_mean, prior_var, noise_var, pm_zero=pm_zero)
    nc = build_nc(pm_zero=pm_zero)
    res = run_bass_kernel_spmd(nc, in_maps, list(range(NCORES)))
    outs = [np.asarray(res.results[c]["out"]).reshape(BL) for c in range(NCORES)]
    return np.concatenate(outs).astype(np.float32)
